# revision 20
# baseline (speedup 1.0000x reference)
"""Trainium2 Bass kernel for nn_Block_14516989461266.

The reference is a 64-step scan where each (b, t) row evolves independently:
    v      = ux + q @ Wm + bm          (ux = x @ Wu + bu, fixed per row)
    s      = clip(set_p * v, 0, 1)
    gate   = mean(s, -1) >= 0.75
    vq     = v @ Wv + bv
    q_new  = vq * gate + q * (1 - gate)
    emits (tanh(v), q_new) each step

Key exact algebraic property: if a row's gate is 0, q is unchanged, so the
next step recomputes the identical v -> identical gate -> fixed point. With
q0 = 0, a row whose first-step gate is 0 emits tanh(ux + bm) and q = 0 for
ALL 64 steps. The device computes only the GEMM v1 = x @ Wu (bf16 inputs,
f32 accumulate); the host adds bu+bm, checks the gate condition, applies
tanh and broadcasts along the step axis. If any gate fires (it does not for
the graded distribution: max mean(s) ~0.17 vs threshold 0.75), a general
host fallback computes the full recurrence.

Sharding: 2 row-halves x 4 U-quarters across the 8 cores (the byte-optimal
256x256 output blocking: each core loads half of x and a quarter of Wu in
bf16, 1 MB total, the minimum possible input for 1/8th of the output).

Schedule (cost-model-driven, 8534 ns): SIX input DMA groups — five hoisted
to the head of SP's stream (SP joins the start barrier after them; PE's
barrier wait is deleted so PE isn't coupled to SP's late arrival) plus ONE
issued from the Pool engine (plain SWDGE DMACopy, hoisted after Pool's
const memset; its ~1.0 us descriptor generation overlaps SP's HWDGE issue
pipeline and bypasses the shared HWDGE, beating the 5-issues-per-engine
limit). The stream [ch0-1 | ch2-3 | ch4-5(Pool) | ch6 | WuA7+x7[0:128] |
x7[128:]+WuB7] is bubble-free 1300..4212 ns; chunk 7 is packed
[WuA7 | x7 | WuB7] so the late groups are contiguous, and the x7 split at
row 128 is forced: both late groups need >= 256 bf16 cols per partition to
avoid the sub-512B DMA descriptor penalty. Late PE batches ride their own
sems ([A6,B6] -> [A7 rows 0:128] -> [A7 rows 128:, B7], waits engine-fused
on the first matmul), so only 160 ns of matmul follows the last input sem:
PE ends 5301 ns. PSUM->SBUF bf16 copies: bank A (retires first) on ACT,
bank B on DVE (GPSIMD cannot access PSUM; splitting a bank across engines
repays the PSUM-access init and loses). One SP output DMA. The end barrier
is deleted entirely: SP's end-block Drain carries the DMA-completion wait
(0 sem receive overhead, retires at the wait), engines halt independently,
and every semaphore ends at 0 so NEFF re-execution sees a clean state.

Cost skeleton (all at model floors): 1300 issue + 2912 stream + 900 sem +
29 wait + 160 final matmuls + ~660 copy chain + 1275 out issue + 364 out
transfer + 900 sem = 8501.
"""

from contextlib import ExitStack

import numpy as np

B, T, D, U = 8, 64, 1024, 1024
NCORES = 8
RSPLIT, CSPLIT = 2, 4          # row halves x U quarters
RR = (B * T) // RSPLIT         # 256 rows per core
UC = U // CSPLIT               # 256 output columns per core (2 PSUM banks)
KC = D // 128                  # 8 contraction chunks of 128
CONSENT = 0.75

# Packed input layout ([128, 4096] bf16 per core):
#   chunks k=0..6 at k*CW: [x_k rows (256) | WuA_k (128) | WuB_k (128)]
#   chunk 7 at 7*CW:       [WuA_7 (128) | x_7 rows (256) | WuB_7 (128)]
CW = RR + 256                  # 512 bf16 cols per chunk
PACK_W = KC * CW               # 4096
# Input DMA groups (column ranges, stream order) and the PE batch gated by
# each. The last two groups split chunk 7's operands: WuA_7 plus the first
# XSPLIT rows of x_7 ride with chunk 6, so batch 3 = [A6, B6, A7[0:XSPLIT]]
# finishes exactly when the final group's (engine-fused) sem admits batch 4
# = [A7[XSPLIT:], B7]. XSPLIT=52 balances b3's end against the 5141 ns gate.
XSPLIT = 128
IN_GROUPS = [
    (0 * CW, 2 * CW),
    (2 * CW, 4 * CW),
    (4 * CW, 6 * CW),                  # Pool-issued (SWDGE, no HWDGE slot)
    (6 * CW, 7 * CW),                  # chunk 6
    (7 * CW, 7 * CW + 128 + XSPLIT),   # WuA_7 + x_7[0:128]
    (7 * CW + 128 + XSPLIT, 8 * CW),   # x_7[128:] + WuB_7
]
POOL_GROUP = 2                         # issued from gpsimd; transfer ready ~2.2us
# PSUM->SBUF copy regions: (engine, bank, row_lo, row_hi). Only ACT and DVE
# can read PSUM (the BIR verifier rejects GPSIMD PSUM access). Bank A (0)
# retires one matmul earlier and goes to ACT (slower engine+sem path); bank
# B (1), the last to retire, goes to DVE whose PSUM access latency and sem
# propagation are lowest. Row-splitting a bank across both engines loses:
# each extra copy pays the full PSUM-access init again.
COPIES = [
    ("scalar", 0, 0, RR),
    ("vector", 1, 0, RR),
]

_CACHE = {}
LAST_RESULTS = None            # BassKernelResults of the most recent device run


def _build_v1_nc():
    """SPMD program: v1 = x_half @ Wu_quarter in bf16, shipped out as bf16.

    Raw Bass (no Tile): this container's walrus build accepts at most ONE
    sync-wait per HW instruction, so each wait_ge is its own sequencer
    instruction.
    """
    import concourse.bass as bass
    import concourse.mybir as mybir

    F32 = mybir.dt.float32
    BF16 = mybir.dt.bfloat16
    nc = bass.Bass()
    xw = nc.dram_tensor("xw", [128, PACK_W], BF16, kind="ExternalInput")
    acts = nc.dram_tensor("acts", [128, 2 * RR], BF16, kind="ExternalOutput")

    n_in = len(IN_GROUPS)
    n_copies = len(COPIES)

    with (
        nc.sbuf_tensor([128, PACK_W], BF16) as xw_t,
        nc.sbuf_tensor([128, 2, RR], BF16) as out_t,
        nc.psum_tensor([128, RR], F32) as ps_a,
        nc.psum_tensor([128, RR], F32) as ps_b,
        nc.psum_tensor([1, RR], F32) as ps_w,
        ExitStack() as _st,
        nc.semaphore("copy_sem") as copy_sem,
        nc.semaphore("out_sem") as out_sem,
        nc.Block(no_gpsimd_drain=True) as block,
    ):
        g_sems = [_st.enter_context(nc.semaphore(f"g{i}")) for i in range(n_in)]
        pe_sems = [_st.enter_context(nc.semaphore(f"pe{i}")) for i in range(n_copies)]
        ps = {0: ps_a, 1: ps_b}

        def wu_ap(k, bank):
            if k == KC - 1:
                off = 0 if bank == 0 else 384
            else:
                off = 256 + bank * 128
            lo = k * CW + off
            return xw_t[:, lo:lo + 128]

        def x_ap(k, r0=0, r1=RR):
            lo = k * CW + (128 if k == KC - 1 else 0)
            return xw_t[:, lo + r0:lo + r1]

        @block.sync
        def _(sync):
            for gi, (lo, hi) in enumerate(IN_GROUPS):
                if gi == POOL_GROUP:
                    continue
                sync.dma_start(xw_t[:, lo:hi], xw[:, lo:hi]).then_inc(g_sems[gi], 16)
            sync.dma_start(acts[:], out_t[:, :, :])._wait_ge(
                copy_sem, n_copies).then_inc(out_sem, 16)

        @block.gpsimd
        def _(gp):
            lo, hi = IN_GROUPS[POOL_GROUP]
            gp.dma_start(xw_t[:, lo:hi], xw[:, lo:hi]).then_inc(
                g_sems[POOL_GROUP], 16)

        # stop-matmuls per copy region: bank A's chunk-7 matmul is row-split
        # in two, so its copy waits for BOTH stop increments.
        REGION_STOPS = {0: 2, 1: 1}

        # copies, grouped per engine in program order
        for eng_name in ("scalar", "vector", "gpsimd"):
            plan = [(i, c) for i, c in enumerate(COPIES) if c[0] == eng_name]
            if not plan:
                continue

            def make(eng, plan=plan, eng_name=eng_name):
                for ci, (_, bank, r0, r1) in plan:
                    dst = out_t[:, bank, r0:r1]
                    src = ps[bank][:, r0:r1]
                    if eng_name == "scalar":
                        op = eng.copy(dst, src)
                    else:
                        op = eng.tensor_copy(dst, src)
                    op._wait_ge(pe_sems[ci], REGION_STOPS[bank]).then_inc(copy_sem, 1)

            getattr(block, eng_name)(make)

        @block.tensor
        def _(tensor):
            # copy regions overlapping a stop-matmul's row range
            def stop_incs(mm, bank):
                for ci, (_, bk, r0, r1) in enumerate(COPIES):
                    if bk == bank:
                        mm.then_inc(pe_sems[ci], 1)

            # Warm-up matmuls: in the cost model they are free (PE is idle
            # until the first group's DMA sem anyway, ~2965 ns), but on real
            # hardware they keep the PE busy from right after the preamble so
            # the p-state ramp isn't reset by the idle gap before batch 0.
            warm_zero = nc.const_aps.aps[(F32, 0.0)].bitcast(BF16)
            for _w in range(11):
                tensor.matmul(
                    ps_w[:, 0:RR], warm_zero[:, 0:1],
                    warm_zero[:, 0:1].to_broadcast((128, RR)),
                    start=True, stop=True,
                )

            # batches: [ch0,1], [ch2,3], [ch4,5], [ch6],
            #          [A7 rows 0:128], [A7 rows 128:, B7]
            # The last three batches' group waits ride on their first matmul
            # (ENGINE-level, 29 ns receive) instead of a standalone wait
            # instruction (~39 ns); fusing earlier batches' waits jams the
            # PE wait queue and simulates much worse.
            batch_chunks = [(0, 1), (2, 3), (4, 5)]
            for gi, chunks in enumerate(batch_chunks):
                tensor.wait_ge(g_sems[gi], 16)
                for k in chunks:
                    for bank in (0, 1):
                        tensor.matmul(
                            ps[bank][:, :], wu_ap(k, bank), x_ap(k),
                            start=(k == 0), stop=False,
                        )
            # b3: [A6, B6] fused on g3
            pend = g_sems[3]
            for bank in (0, 1):
                mm = tensor.matmul(
                    ps[bank][:, :], wu_ap(6, bank), x_ap(6),
                    start=False, stop=False,
                )
                if pend is not None:
                    mm._wait_ge(pend, 16)
                    pend = None
            # b4: [A7 rows 0:XSPLIT] fused on g4
            mm = tensor.matmul(
                ps[0][:, 0:XSPLIT], wu_ap(KC - 1, 0), x_ap(KC - 1, 0, XSPLIT),
                start=False, stop=True,
            )
            mm._wait_ge(g_sems[4], 16)
            stop_incs(mm, 0)
            # b5: [A7 rows XSPLIT:, B7] fused on g5
            mm = tensor.matmul(
                ps[0][:, XSPLIT:RR], wu_ap(KC - 1, 0), x_ap(KC - 1, XSPLIT, RR),
                start=False, stop=True,
            )
            mm._wait_ge(g_sems[5], 16)
            stop_incs(mm, 0)
            mm = tensor.matmul(
                ps[1][:, :], wu_ap(KC - 1, 1), x_ap(KC - 1),
                start=False, stop=True,
            )
            stop_incs(mm, 1)

    # --- post-IR surgery -------------------------------------------------
    # 1) The framework preamble memsets four const-AP tensors on the Pool
    # engine and every engine's start barrier waits for them. Only
    # const-float32-0.0 is ever read (the activation engine's implicit zero
    # bias for the scalar copy); dropping the other three memsets moves the
    # whole schedule earlier.
    keep = {"const-float32-0.0"}
    blk0 = nc.m.functions[0].blocks[0]
    pruned = []
    for inst in blk0.instructions:
        if isinstance(inst, mybir.InstMemset):
            try:
                name = inst.outs[0].bass_ap.tensor.name
            except AttributeError:
                name = ""
            if name.startswith("const-") and name not in keep:
                continue
        pruned.append(inst)
    blk0.instructions[:] = pruned

    # 2) Hoist ALL input DMAs to the head of SP's stream, with SP's start-
    # barrier participation (Drain inc + release wait) AFTER them. The DMAs
    # read no registers and touch tensors nothing in the preamble reads, and
    # their completion sems fire long after the barrier resolves, so issuing
    # them pre-barrier is order-safe. To keep PE from stalling on SP's late
    # barrier arrival, PE's barrier-release wait is deleted and Pool's
    # release post dropped from 4 to 3 (ACT/DVE/SP still wait-and-dec, so
    # sem 152 returns to 0 for the end barrier's eq-0 waits).
    SP = mybir.EngineType.SP
    PE = mybir.EngineType.PE
    in_dmas = []
    for blk in nc.m.functions[0].blocks[1:]:
        for inst in list(blk.instructions):
            if isinstance(inst, mybir.InstDMACopy) and inst.engine == SP:
                try:
                    src_name = inst.ins[0].bass_ap.tensor.name
                except AttributeError:
                    src_name = ""
                if src_name == "xw":
                    in_dmas.append(inst)
                    blk.instructions.remove(inst)
    assert len(in_dmas) == n_in - 1  # all but the Pool-issued group
    sp_insts = [inst for inst in blk0.instructions if inst.engine == SP]
    regmoves = [i for i in sp_insts if isinstance(i, mybir.InstRegisterMove)]
    drain = next(i for i in sp_insts if isinstance(i, mybir.InstDrain))
    evsem = next(i for i in sp_insts if isinstance(i, mybir.InstEventSemaphore))
    branch = [i for i in sp_insts if isinstance(i, mybir.InstUnconditionalBranch)]
    new_sp = in_dmas + [drain, evsem] + regmoves + branch

    pe_barrier_wait = next(
        inst for inst in blk0.instructions
        if inst.engine == PE and isinstance(inst, mybir.InstEventSemaphore)
    )
    pool_post0 = next(
        inst for inst in blk0.instructions
        if inst.engine == mybir.EngineType.Pool
        and isinstance(inst, mybir.InstEventSemaphore)
        and inst.sync_info is not None
        and any(u.update_mode == "sem-add-imm" and u.update_value == 4
                for u in inst.sync_info.on_update)
    )
    for u in pool_post0.sync_info.on_update:
        if u.update_mode == "sem-add-imm" and u.update_value == 4:
            u.update_value = 3
    assert any(u.update_value == 3 for u in pool_post0.sync_info.on_update)

    others = [inst for inst in blk0.instructions
              if inst.engine != SP and inst is not pe_barrier_wait]
    blk0.instructions[:] = new_sp + others

    # Hoist the Pool-issued input DMA into block 0 right after Pool's const
    # memset: its SWDGE descriptor generation (~1.0 us on the Pool engine)
    # then overlaps SP's HWDGE issue pipeline and its transfer is ready in
    # time to carry the stream's third slot. Pool's barrier collect comes
    # after, so cross-engine ordering is unchanged.
    Pool = mybir.EngineType.Pool
    pool_dma = None
    for blk in nc.m.functions[0].blocks[1:]:
        for inst in list(blk.instructions):
            if isinstance(inst, mybir.InstDMACopy) and inst.engine == Pool:
                try:
                    src_name = inst.ins[0].bass_ap.tensor.name
                except AttributeError:
                    src_name = ""
                if src_name == "xw":
                    pool_dma = inst
                    blk.instructions.remove(inst)
    assert pool_dma is not None
    memset_i = next(
        i for i, inst in enumerate(blk0.instructions)
        if inst.engine == Pool and isinstance(inst, mybir.InstMemset)
    )
    blk0.instructions.insert(memset_i + 1, pool_dma)

    # 3) End barrier: replace it entirely with a single completion waiter —
    # SP's end-block Drain carries the output-DMA-completion wait (SP SEQ
    # has 0 sem receive overhead, and the Drain retires at the wait itself).
    # All end-block EventSemaphores (barrier arrivals/collect/release) are
    # deleted: engines halt independently, SP halts last and happens-after
    # the DRAM write, and every semaphore ends the program at 0 so a repeat
    # execution of the same NEFF sees a clean initial state.
    import bass_rust as _bass_rust
    end_blk = nc.m.functions[0].blocks[-1]
    sp_drain = next(
        inst for inst in end_blk.instructions
        if inst.engine == SP and isinstance(inst, mybir.InstDrain)
    )
    _bass_rust.wait_op(sp_drain, out_sem, 16, "sem-ge", True)
    kill = [
        inst for inst in end_blk.instructions
        if isinstance(inst, mybir.InstEventSemaphore)
    ]
    for inst in kill:
        end_blk.instructions.remove(inst)

    return nc


def _pack_core_inputs(x2d, Wu):
    """Per-core packed [128, PACK_W] bf16 inputs for all 8 cores."""
    import ml_dtypes

    bf16 = ml_dtypes.bfloat16
    R = B * T
    # x chunks transposed: xt[k] = x2d[:, k*128:(k+1)*128].T  -> [128, R]
    xt = np.ascontiguousarray(x2d.T.reshape(KC, 128, R)).astype(bf16)
    Wub = Wu.astype(bf16)

    in_maps = []
    for core in range(NCORES):
        rh, cq = divmod(core, CSPLIT)
        xw = np.empty((128, PACK_W), bf16)
        xpart = xt[:, :, rh * RR:(rh + 1) * RR]          # [KC, 128, RR]
        for k in range(KC):
            base = k * CW
            wa = Wub[k * 128:(k + 1) * 128, cq * UC:cq * UC + 128]
            wb = Wub[k * 128:(k + 1) * 128, cq * UC + 128:(cq + 1) * UC]
            if k == KC - 1:
                xw[:, base:base + 128] = wa
                xw[:, base + 128:base + 384] = xpart[k]
                xw[:, base + 384:base + 512] = wb
            else:
                xw[:, base:base + RR] = xpart[k]
                xw[:, base + RR:base + 384] = wa
                xw[:, base + 384:base + 512] = wb
        in_maps.append({"xw": xw})
    return in_maps


def _run_v1_kernel(x2d, Wu):
    """Run the SPMD kernel. Returns v1 = x2d @ Wu as [R, U] float32."""
    from concourse.bass_utils import run_bass_kernel_spmd

    global LAST_RESULTS
    if "v1" not in _CACHE:
        _CACHE["v1"] = _build_v1_nc()
    nc = _CACHE["v1"]

    in_maps = _pack_core_inputs(x2d, Wu)
    res = run_bass_kernel_spmd(nc, in_maps, list(range(NCORES)))
    LAST_RESULTS = res

    R = B * T
    v1 = np.empty((R, U), np.float32)
    for core in range(NCORES):
        rh, cq = divmod(core, CSPLIT)
        a = np.asarray(res.results[core]["acts"])           # [128, 2*RR] bf16
        a = a.reshape(128, 2, RR).transpose(1, 0, 2)        # [2, 128, RR]
        block = a.reshape(UC, RR).astype(np.float32).T      # [RR, UC]
        v1[rh * RR:(rh + 1) * RR, cq * UC:(cq + 1) * UC] = block
    return v1


def _fallback_full_scan(x2d, Wu, bu, Wm, bm, Wv, bv, set_p):
    """General-input path: the full 64-step recurrence (numpy, fp32)."""
    R = B * T
    ux = (x2d @ Wu + bu).astype(np.float32)
    q = np.zeros_like(ux)
    acts = np.empty((T, R, U), np.float32)
    qs = np.empty((T, R, U), np.float32)
    for step in range(T):
        v = (ux + q @ Wm + bm).astype(np.float32)
        s = np.clip(set_p * v, 0.0, 1.0)
        gate = (s.mean(axis=-1) >= CONSENT).astype(np.float32)[:, None]
        vq = (v @ Wv + bv).astype(np.float32)
        q = vq * gate + q * (1.0 - gate)
        acts[step] = np.tanh(v)
        qs[step] = q
    acts = acts.reshape(T, B, T, U).transpose(1, 0, 2, 3)
    qs = qs.reshape(T, B, T, U).transpose(1, 0, 2, 3)
    return np.ascontiguousarray(acts), np.ascontiguousarray(qs)


def kernel(x, Wu, bu, Wm, bm, Wv, bv, set_p):
    x = np.asarray(x, np.float32)
    Wu = np.asarray(Wu, np.float32)
    bu = np.asarray(bu, np.float32)
    Wm = np.asarray(Wm, np.float32)
    bm = np.asarray(bm, np.float32)
    Wv = np.asarray(Wv, np.float32)
    bv = np.asarray(bv, np.float32)
    set_p = np.asarray(set_p, np.float32)

    x2d = np.ascontiguousarray(x.reshape(B * T, D))
    bub = (bu + bm).astype(np.float32)

    try:
        v1 = _run_v1_kernel(x2d, Wu)
    except Exception as e:  # infrastructure failure only -- not data-driven
        print(f"WARNING: Trainium path failed ({type(e).__name__}: {e}); "
              "computing the full recurrence on host instead.")
        return _fallback_full_scan(x2d, Wu, bu, Wm, bm, Wv, bv, set_p)

    v1 = v1 + bub
    s = np.clip(set_p * v1, 0.0, 1.0)
    if np.any(s.mean(axis=-1) >= CONSENT):
        # Some row latches at step 1 -> the fixed-point shortcut is invalid;
        # compute the general recurrence.
        return _fallback_full_scan(x2d, Wu, bu, Wm, bm, Wv, bv, set_p)

    # No gate fires at step 1 with q0 = 0 -> q stays 0 and every step
    # emits the identical tanh(v1): broadcast along the step axis.
    act1 = np.tanh(v1).reshape(B, 1, T, U)
    acts = np.empty((B, T, T, U), np.float32)
    acts[:] = act1
    qs = np.zeros((B, T, T, U), np.float32)
    return acts, qs
